# revision 1
# baseline (speedup 1.0000x reference)
"""Trainium2 Bass kernel for C = triu(triu(A) @ triu(B)), N=4096, fp32.

Math: the product of upper-triangular matrices is upper-triangular, so with
host-side triu masking of A and B the kernel output needs no masking: for an
output tile (m, n) (128x128 tile indices), the contraction over k only gets
contributions from k in [m, n]; tiles below the diagonal are exactly zero.

Sharding (8 cores, SPMD, one NEFF): block-cyclic rows. Core j owns the four
128-row tiles {j, 8+j, 16+j, 24+j} of A and C (512 rows per core); B
(triu-masked) is replicated. All cores run the identical program; where the
program's k-range extends past a core's actual triangle the masked A columns
are zero, so the extra matmuls accumulate zeros and stay correct. This makes
the per-core instruction streams (and hence runtimes) identical by
construction - no load imbalance.

Per-core program: the transposed A shard (lhsT layout, [128, 32, 512]) stays
SBUF-resident; B is streamed tile-by-tile ([128, 512], each tile touched
exactly once); C accumulates in PSUM banks (up to 4 live), is copied out via
VectorE and DMAed to DRAM.

Loop structure: for each 512-wide column super-block s (8 of them), for each
k-tile <= 4s+3, load B[k, s-block] once and matmul it against the A tiles of
every owned row-slot t with 8t <= k, accumulating into psum[t].
"""

import os
import sys

for _p in ("/opt/trn_rl_repo", "/root/.axon_site/_ro/trn_rl_repo"):
    if _p not in sys.path:
        sys.path.insert(0, _p)

import numpy as np

N = 4096
P = 128
NCORES = 8
NSLOT = 4  # row-tiles per core
SW = 512  # n super-block width
NS = N // SW  # 8 supers
KT = N // P  # 32 k-tiles

# matmul input dtype: "f32" (exact, 4 cyc/row), "f32r" (tf32-like, 1 cyc/row
# at free dim >= 256), "bf16" (1 cyc/row, half the DMA traffic)
MM_DTYPE = os.environ.get("MM_DTYPE", "bf16")

_cache = {}


def _build(dt_mode):
    import concourse.bacc as bacc
    import concourse.mybir as mybir
    import concourse.tile as tile

    D = {
        "f32": mybir.dt.float32,
        "f32r": mybir.dt.float32r,
        "bf16": mybir.dt.bfloat16,
    }[dt_mode]

    nc = bacc.Bacc(None, target_bir_lowering=False)
    AT = nc.dram_tensor("AT", [P, KT, NSLOT * P], D, kind="ExternalInput")
    # B packed per n-super: B_packed[s, p, ko, w] = triu(B)[128*ko + p, 512*s + w]
    # so a k-chunk load is per-partition contiguous (KCHUNK*512 elements).
    Bm = nc.dram_tensor("B", [NS, P, KT, SW], D, kind="ExternalInput")
    Cm = nc.dram_tensor("C", [NSLOT * P, N], mybir.dt.float32, kind="ExternalOutput")

    KCHUNK = 4
    b_bufs = 12 if dt_mode == "bf16" else 6

    with tile.TileContext(nc) as tc:
        with (
            tc.tile_pool(name="a", bufs=4) as apool,
            tc.tile_pool(name="b", bufs=b_bufs) as bpool,
            tc.tile_pool(name="o", bufs=4) as opool,
            tc.tile_pool(name="ps", bufs=8, space="PSUM") as pspool,
        ):
            # A shard resident in 4 independent tiles so early matmuls only
            # wait on the first chunk
            # A loads go on the Scalar engine's DMA queue so they stream in
            # parallel with the B chunks issued from the Sync queue
            a_tiles = []
            for g in range(4):
                ag = apool.tile([P, 8, NSLOT * P], D, tag=f"a{g}", name="ag")
                nc.scalar.dma_start(ag[:], AT[:, 8 * g : 8 * (g + 1), :])
                a_tiles.append(ag)

            for s in range(NS):
                kmax = 4 * s + 3
                nslots = kmax // 8 + 1
                psums = [
                    pspool.tile([P, SW], mybir.dt.float32, tag="ps", name="ps")
                    for _ in range(nslots)
                ]
                for kc in range(0, kmax + 1, KCHUNK):
                    cnt = min(KCHUNK, kmax + 1 - kc)
                    bt = bpool.tile([P, KCHUNK, SW], D, tag="b", name="bt")
                    nc.sync.dma_start(bt[:, :cnt, :], Bm[s, :, kc : kc + cnt, :])
                    for k in range(kc, kc + cnt):
                        # columns left of 128*(k - 4s) are k < n-tile regions
                        # where triu(B) is zero; skip them
                        w0 = max(0, P * (k - 4 * s))
                        for t in range(k // 8 + 1):
                            nc.tensor.matmul(
                                psums[t][:, w0:SW],
                                a_tiles[k // 8][:, k % 8, P * t : P * (t + 1)],
                                bt[:, k - kc, w0:SW],
                                start=(k == 8 * t),
                                stop=(k == kmax),
                            )
                for t in range(nslots):
                    ot = opool.tile([P, SW], mybir.dt.float32, tag="o", name="ot")
                    nc.vector.tensor_copy(ot[:], psums[t][:])
                    # C stores on the GpSimd queue: keeps the Sync queue free
                    # for B streaming
                    nc.gpsimd.dma_start(
                        Cm[P * t : P * (t + 1), SW * s : SW * (s + 1)], ot[:]
                    )
    nc.compile()
    return nc


def _get_nc():
    if MM_DTYPE not in _cache:
        _cache[MM_DTYPE] = _build(MM_DTYPE)
    return _cache[MM_DTYPE]


def _np_dtype():
    if MM_DTYPE == "bf16":
        import ml_dtypes

        return np.dtype(ml_dtypes.bfloat16)
    return np.dtype(np.float32)


def _make_in_maps(A, B):
    A = np.asarray(A, dtype=np.float32)
    B = np.asarray(B, dtype=np.float32)
    Au = np.triu(A)
    Bu = np.triu(B)

    npdt = _np_dtype()
    # pack: B_packed[s, p, ko, w] = Bu[128*ko + p, 512*s + w]
    Bu_c = np.ascontiguousarray(
        Bu.reshape(KT, P, NS, SW).transpose(2, 1, 0, 3)
    )
    if npdt != np.float32:
        Bu_c = Bu_c.astype(npdt)

    in_maps = []
    for j in range(NCORES):
        rows = np.concatenate(
            [
                np.arange(P * (NCORES * t + j), P * (NCORES * t + j) + P)
                for t in range(NSLOT)
            ]
        )
        A_loc = Au[rows, :]  # [512, 4096]
        # lhsT layout [p, ko, ml]: element = A_loc[ml, ko*128 + p]
        ATd = np.ascontiguousarray(
            A_loc.reshape(NSLOT * P, KT, P).transpose(2, 1, 0)
        )
        if npdt != np.float32:
            ATd = ATd.astype(npdt)
        in_maps.append({"AT": ATd, "B": Bu_c})
    return in_maps


def kernel(A, B):
    from concourse.bass_utils import run_bass_kernel_spmd

    in_maps = _make_in_maps(A, B)
    nc = _get_nc()
    res = run_bass_kernel_spmd(nc, in_maps, core_ids=list(range(NCORES)))

    C = np.zeros((N, N), dtype=np.float32)
    for j in range(NCORES):
        Cj = res.results[j]["C"]
        for t in range(NSLOT):
            m = NCORES * t + j
            C[P * m : P * (m + 1), :] = Cj[P * t : P * (t + 1), :]
    return C



# revision 2
# speedup vs baseline: 1.2703x; 1.2703x over previous
"""Trainium2 Bass kernel for C = triu(triu(A) @ triu(B)), N=4096, fp32.

Math: with host-side triu masking of A and B, the product of upper-triangular
matrices is upper-triangular; for an output tile (m, n) (128x128 tile indices)
the contraction over k only gets contributions from k in [m, n].

Sharding (8 cores, SPMD, one NEFF): 2D grid, 4 row groups x 2 column groups.
Core j = (r = j%4, c = j//4):
  - rows:    core owns row-tiles m with m % 4 == r  (8 slots, 1024 rows)
  - columns: core owns col-tiles n with n % 2 == c  (16 tiles, 2048 cols),
             gathered into 4 local 512-wide supers u: n in {8u+c+2j, j=0..3}
This cuts per-core HBM traffic from ~28MB (replicated-B row sharding) to
~16MB: A is triangle-packed (4.7MB bf16), B is quarter.. half-sharded with
diagonal trimming (~8.4MB), C is stored as bf16 (2.6MB).

All cores run the identical program. Where a core's actual triangle is
smaller than the program's loop bounds (k-start 4t vs true row 4t+r; column
group c=0 vs the c=1 loop shapes) the host-packed operands hold zeros, so the
extra matmuls accumulate zeros and stay correct.

Per-core program: packed A (lhsT layout, [128, 144, 128]) is SBUF-resident;
B supers are streamed (double-buffered, 4-ktile chunks); loops are
slot-major within a super so at most ~2 PSUM banks are live and each output
tile drains (vector copy to bf16 + DMA) while the next slot computes. Supers
run descending (u=3 first) so the serialized tail is the tiny u=0 super.
"""

import os
import sys

for _p in ("/opt/trn_rl_repo", "/root/.axon_site/_ro/trn_rl_repo"):
    if _p not in sys.path:
        sys.path.insert(0, _p)

import numpy as np

N = 4096
P = 128
KT = 32  # k tiles
GR = 4  # row groups
GC = 2  # col groups
NSLOT = 8  # row slots per core
NSUP = 4  # local 512-wide supers per core
SW = 512
NCORES = 8

# A pack: pairs (t, ko) with ko in [4t, 31], t-major
A_OFF = [0] * NSLOT
for _t in range(1, NSLOT):
    A_OFF[_t] = A_OFF[_t - 1] + (KT - 4 * (_t - 1))
NPAIR = A_OFF[-1] + (KT - 4 * (NSLOT - 1))  # 144

NCT = sum(2 * u + 2 for u in range(NSUP))  # 20 C tiles per core


def _c_off(u):
    return u * u + u


def _w0(ko, u):
    # leftmost nonzero column (c=1 core) of local super u at k-tile ko
    return 128 * max(0, min(3, (ko - 8 * u) // 2))


# C store dtype: bf16 halves store traffic; rel-err budget (2e-2) dominated
# by bf16 matmul inputs either way
C_DTYPE = os.environ.get("C_DTYPE", "bf16")

_cache = {}


def _build(c_dtype):
    import concourse.bacc as bacc
    import concourse.mybir as mybir
    import concourse.tile as tile

    D = mybir.dt.bfloat16
    DC = mybir.dt.bfloat16 if c_dtype == "bf16" else mybir.dt.float32

    nc = bacc.Bacc(None, target_bir_lowering=False)
    ATp = nc.dram_tensor("ATp", [P, NPAIR, P], D, kind="ExternalInput")
    Bp = nc.dram_tensor("B", [NSUP, P, KT, SW], D, kind="ExternalInput")
    Cp = nc.dram_tensor("C", [NCT, P, SW], DC, kind="ExternalOutput")

    with tile.TileContext(nc) as tc:
        with (
            tc.tile_pool(name="a", bufs=1) as apool,
            tc.tile_pool(name="b", bufs=2) as bpool,
            tc.tile_pool(name="o", bufs=4) as opool,
            tc.tile_pool(name="ps", bufs=6, space="PSUM") as pspool,
        ):
            # resident A, streamed in chunks on the Scalar queue so it loads
            # in parallel with B chunks on the Sync queue
            a_res = apool.tile([P, NPAIR, P], D, tag="a", name="ar")
            ACH = 24
            for c0 in range(0, NPAIR, ACH):
                nc.scalar.dma_start(
                    a_res[:, c0 : c0 + ACH, :], ATp[:, c0 : c0 + ACH, :]
                )

            for u in range(NSUP - 1, -1, -1):
                kmaxu = 8 * u + 7
                bt = bpool.tile([P, KT, SW], D, tag="b", name="bt")
                for kc in range(0, kmaxu + 1, 4):
                    # last chunk crosses the diagonal: left 256 cols of its
                    # k-tiles are (mostly) zero in triu(B); skip loading them
                    w0c = 256 if kc == 8 * u + 4 else 0
                    nc.sync.dma_start(
                        bt[:, kc : kc + 4, w0c:], Bp[u, :, kc : kc + 4, w0c:]
                    )
                for t in range(2 * u + 2):
                    ps = pspool.tile([P, SW], mybir.dt.float32, tag="ps", name="ps")
                    for ko in range(4 * t, kmaxu + 1):
                        w0 = _w0(ko, u)
                        nc.tensor.matmul(
                            ps[:, w0:],
                            a_res[:, A_OFF[t] + ko - 4 * t, :],
                            bt[:, ko, w0:],
                            start=(ko == 4 * t),
                            stop=(ko == kmaxu),
                        )
                    ot = opool.tile([P, SW], DC, tag="o", name="ot")
                    nc.vector.tensor_copy(ot[:], ps[:])
                    # C stores on the GpSimd queue keep Sync free for B
                    nc.gpsimd.dma_start(Cp[_c_off(u) + t], ot[:])
    nc.compile()
    return nc


def _get_nc():
    if C_DTYPE not in _cache:
        _cache[C_DTYPE] = _build(C_DTYPE)
    return _cache[C_DTYPE]


def _make_in_maps(A, B):
    import ml_dtypes

    bf16 = np.dtype(ml_dtypes.bfloat16)
    A32 = np.asarray(A, dtype=np.float32)
    B32 = np.asarray(B, dtype=np.float32)
    Au = np.triu(A32).astype(bf16)
    Bu = np.triu(B32).astype(bf16)

    # B pack per column group c:
    #   Bp[u, p, ko, 128*j + wc] = Bu[128*ko + p, 128*(8u + c + 2j) + wc]
    Xb = Bu.reshape(KT, P, KT, P)
    Bpacks = []
    for c in range(GC):
        Bp = np.stack(
            [
                np.ascontiguousarray(
                    Xb[:, :, 8 * u + c : 8 * u + c + 8 : 2, :]
                    .transpose(1, 0, 2, 3)
                    .reshape(P, KT, SW)
                )
                for u in range(NSUP)
            ]
        )
        Bpacks.append(Bp)

    # A pack per row group r: lhsT pairs (t, ko), ko in [4t, 31]:
    #   ATp[p, A_OFF[t] + ko - 4t, ml] = Au[128*(4t+r) + ml, 128*ko + p]
    ATpacks = []
    for r in range(GR):
        blocks = []
        for t in range(NSLOT):
            m = 4 * t + r
            blk = Au[P * m : P * (m + 1), P * 4 * t :]
            blk = blk.reshape(P, KT - 4 * t, P).transpose(2, 1, 0)
            blocks.append(blk)
        ATpacks.append(np.ascontiguousarray(np.concatenate(blocks, axis=1)))

    in_maps = []
    for j in range(NCORES):
        r, c = j % GR, j // GR
        in_maps.append({"ATp": ATpacks[r], "B": Bpacks[c]})
    return in_maps


def kernel(A, B):
    from concourse.bass_utils import run_bass_kernel_spmd

    in_maps = _make_in_maps(A, B)
    nc = _get_nc()
    res = run_bass_kernel_spmd(nc, in_maps, core_ids=list(range(NCORES)))

    C = np.zeros((N, N), dtype=np.float32)
    for j in range(NCORES):
        r, c = j % GR, j // GR
        Cj = res.results[j]["C"]
        for u in range(NSUP):
            for t in range(2 * u + 2):
                m = 4 * t + r
                tile_ = Cj[_c_off(u) + t]
                for jj in range(4):
                    n = 8 * u + c + 2 * jj
                    if n >= m:
                        C[P * m : P * (m + 1), P * n : P * (n + 1)] = tile_[
                            :, P * jj : P * (jj + 1)
                        ].astype(np.float32)
    return C


# revision 6
# speedup vs baseline: 1.2804x; 1.0079x over previous
"""Trainium2 Bass kernel for C = triu(triu(A) @ triu(B)), N=4096, fp32.

Math: with host-side triu masking of A and B, the product of upper-triangular
matrices is upper-triangular; for an output tile (m, n) (128x128 tile indices)
the contraction over k only gets contributions from k in [m, n].

Sharding (8 cores, SPMD, one NEFF): 2D grid, 4 row groups x 2 column groups.
Core j = (r = j%4, c = j//4):
  - rows:    core owns row-tiles m with m % 4 == r  (8 slots, 1024 rows)
  - columns: core owns col-tiles n with n % 2 == c  (16 tiles, 2048 cols),
             gathered into 4 local 512-wide supers u: n in {8u+c+2j, j=0..3}
This cuts per-core HBM traffic from ~28MB (replicated-B row sharding) to
~16MB: A is triangle-packed (4.7MB bf16), B is quarter.. half-sharded with
diagonal trimming (~8.4MB), C is stored as bf16 (2.6MB).

All cores run the identical program. Where a core's actual triangle is
smaller than the program's loop bounds (k-start 4t vs true row 4t+r; column
group c=0 vs the c=1 loop shapes) the host-packed operands hold zeros, so the
extra matmuls accumulate zeros and stay correct.

Per-core program: packed A (lhsT layout, [128, 144, 128]) is SBUF-resident;
B supers are streamed (double-buffered, 4-ktile chunks); loops are
slot-major within a super so at most ~2 PSUM banks are live and each output
tile drains (vector copy to bf16 + DMA) while the next slot computes. Supers
run descending (u=3 first) so the serialized tail is the tiny u=0 super.
"""

import os
import sys

for _p in ("/opt/trn_rl_repo", "/root/.axon_site/_ro/trn_rl_repo"):
    if _p not in sys.path:
        sys.path.insert(0, _p)

import numpy as np

N = 4096
P = 128
KT = 32  # k tiles
GR = 4  # row groups
GC = 2  # col groups
NSLOT = 8  # row slots per core
NSUP = 4  # local 512-wide supers per core
SW = 512
NCORES = 8

# A pack: pairs (t, ko) with ko in [4t, 31], ko-major (matches the ko-major
# consumption order of the compute loops, so A streams in exactly as needed)
A_POS = {}
_p = 0
for _ko in range(KT):
    for _t in range(min(_ko // 4, NSLOT - 1) + 1):
        A_POS[(_t, _ko)] = _p
        _p += 1
NPAIR = _p  # 144

NCT = sum(2 * u + 2 for u in range(NSUP))  # 20 C tiles per core


def _c_off(u):
    return u * u + u


def _w0(ko, u):
    # leftmost nonzero column (c=1 core) of local super u at k-tile ko
    return 128 * max(0, min(3, (ko - 8 * u) // 2))


# C store dtype: bf16 halves store traffic; rel-err budget (2e-2) dominated
# by bf16 matmul inputs either way
C_DTYPE = os.environ.get("C_DTYPE", "bf16")

_cache = {}


def _build(c_dtype):
    import concourse.bacc as bacc
    import concourse.mybir as mybir
    import concourse.tile as tile

    D = mybir.dt.bfloat16
    DC = mybir.dt.bfloat16 if c_dtype == "bf16" else mybir.dt.float32

    nc = bacc.Bacc(None, target_bir_lowering=False)
    ATp = nc.dram_tensor("ATp", [P, NPAIR, P], D, kind="ExternalInput")
    Bp = nc.dram_tensor("B", [NSUP, P, KT, SW], D, kind="ExternalInput")
    Cp = nc.dram_tensor("C", [NCT, P, SW], DC, kind="ExternalOutput")

    with tile.TileContext(nc) as tc:
        with (
            tc.tile_pool(name="a", bufs=1) as apool,
            tc.tile_pool(name="b", bufs=2) as bpool,
            tc.tile_pool(name="o", bufs=4) as opool,
            tc.tile_pool(name="ps", bufs=8, space="PSUM") as pspool,
        ):
            # resident A, streamed in chunks on the Scalar queue so it loads
            # in parallel with B chunks on the Sync queue; first chunk small
            # so the first matmul starts early
            a_res = apool.tile([P, NPAIR, P], D, tag="a", name="ar")
            a_edges = [0, 12, 36, 68, 100, NPAIR]
            for c0, c1 in zip(a_edges, a_edges[1:]):
                nc.scalar.dma_start(a_res[:, c0:c1, :], ATp[:, c0:c1, :])

            for u in range(NSUP - 1, -1, -1):
                kmaxu = 8 * u + 7
                nslots = 2 * u + 2
                bt = bpool.tile([P, KT, SW], D, tag="b", name="bt")
                b_edges = [0, 2, 4] if u == NSUP - 1 else [0, 4]
                while b_edges[-1] < kmaxu + 1:
                    b_edges.append(b_edges[-1] + 4)
                for kc0, kc1 in zip(b_edges, b_edges[1:]):
                    # last chunk crosses the diagonal: left 256 cols of its
                    # k-tiles are (mostly) zero in triu(B); skip loading them
                    w0c = 256 if kc0 == 8 * u + 4 else 0
                    nc.sync.dma_start(
                        bt[:, kc0:kc1, w0c:], Bp[u, :, kc0:kc1, w0c:]
                    )
                # ko-major: each B k-tile feeds every eligible row slot
                # back-to-back, so the tensor engine consumes B no faster
                # than ~2x the stream rate and never starves mid-super
                psums = [
                    pspool.tile([P, SW], mybir.dt.float32, tag="ps", name="ps")
                    for _ in range(nslots)
                ]
                for ko in range(kmaxu + 1):
                    w0 = _w0(ko, u)
                    for t in range(min(ko // 4, nslots - 1) + 1):
                        nc.tensor.matmul(
                            psums[t][:, w0:],
                            a_res[:, A_POS[(t, ko)], :],
                            bt[:, ko, w0:],
                            start=(ko == 4 * t),
                            stop=(ko == kmaxu),
                        )
                for t in range(nslots):
                    ot = opool.tile([P, SW], DC, tag="o", name="ot")
                    nc.vector.tensor_copy(ot[:], psums[t][:])
                    # C stores on the GpSimd queue keep Sync free for B
                    nc.gpsimd.dma_start(Cp[_c_off(u) + t], ot[:])
    nc.compile()
    return nc


def _get_nc():
    if C_DTYPE not in _cache:
        _cache[C_DTYPE] = _build(C_DTYPE)
    return _cache[C_DTYPE]


def _make_in_maps(A, B):
    import ml_dtypes

    bf16 = np.dtype(ml_dtypes.bfloat16)
    A32 = np.asarray(A, dtype=np.float32)
    B32 = np.asarray(B, dtype=np.float32)
    Au = np.triu(A32).astype(bf16)
    Bu = np.triu(B32).astype(bf16)

    # B pack per column group c:
    #   Bp[u, p, ko, 128*j + wc] = Bu[128*ko + p, 128*(8u + c + 2j) + wc]
    Xb = Bu.reshape(KT, P, KT, P)
    Bpacks = []
    for c in range(GC):
        Bp = np.stack(
            [
                np.ascontiguousarray(
                    Xb[:, :, 8 * u + c : 8 * u + c + 8 : 2, :]
                    .transpose(1, 0, 2, 3)
                    .reshape(P, KT, SW)
                )
                for u in range(NSUP)
            ]
        )
        Bpacks.append(Bp)

    # A pack per row group r: lhsT pairs (t, ko), ko-major order:
    #   ATp[p, A_POS[(t, ko)], ml] = Au[128*(4t+r) + ml, 128*ko + p]
    ATpacks = []
    for r in range(GR):
        ATp = np.empty((P, NPAIR, P), dtype=bf16)
        for (t, ko), pos in A_POS.items():
            m = 4 * t + r
            ATp[:, pos, :] = Au[P * m : P * (m + 1), P * ko : P * (ko + 1)].T
        ATpacks.append(ATp)

    in_maps = []
    for j in range(NCORES):
        r, c = j % GR, j // GR
        in_maps.append({"ATp": ATpacks[r], "B": Bpacks[c]})
    return in_maps


def kernel(A, B):
    from concourse.bass_utils import run_bass_kernel_spmd

    in_maps = _make_in_maps(A, B)
    nc = _get_nc()
    res = run_bass_kernel_spmd(nc, in_maps, core_ids=list(range(NCORES)))

    C = np.zeros((N, N), dtype=np.float32)
    for j in range(NCORES):
        r, c = j % GR, j // GR
        Cj = res.results[j]["C"]
        for u in range(NSUP):
            for t in range(2 * u + 2):
                m = 4 * t + r
                tile_ = Cj[_c_off(u) + t]
                for jj in range(4):
                    n = 8 * u + c + 2 * jj
                    if n >= m:
                        C[P * m : P * (m + 1), P * n : P * (n + 1)] = tile_[
                            :, P * jj : P * (jj + 1)
                        ].astype(np.float32)
    return C
